# revision 13
# baseline (speedup 1.0000x reference)
"""Bass/Trainium2 kernel for nn_BlastocystAuxLoss.

Computes a masked MSE over B=16,777,216 elements:
    late stages are labels 8..15; target[s] = (s-8) * 4/7 for late stages;
    loss = sum_{s>=8} (x - target)^2 / count(s>=8)   (0.0 if count == 0)

Strategy: trivially data-parallel over 8 NeuronCores. Each core reads its
B/8 shard of blast_scores (f32) and stage_labels (i32) from HBM, computes
per-partition partial {count, sse} on-chip (DVE + ACT engines, bf16
elementwise math, f32 accumulation), and writes a [128, 2] partial-sums
tile. The final scalar reduction (8*128 partials -> sse/cnt) happens on
host in f64. No collectives needed.

Per-element identities used (s = label, x = score):
    mask  m = (s >= 8)
    target t = relu(s * 4/7 - 32/7)        (== (s-8)*4/7 clamped at 0)
    sse  += (m * (bf16(x) - t))^2          (m^2 == m)
    cnt  += m
"""

import numpy as np

B = 16777216
N_CORES = 8
SHARD = B // N_CORES  # 2,097,152
P = 128

_NC_CACHE = {}


def build(shard=SHARD, n_tiles=8):
    """Build the single-core Bass program (same SPMD program for all cores)."""
    import concourse.bacc as bacc
    import concourse.tile as tile
    from concourse import mybir

    free = shard // P
    fd = free // n_tiles
    assert fd * n_tiles * P == shard

    nc = bacc.Bacc("TRN2", target_bir_lowering=False)
    x_ext = nc.declare_dram_parameter(
        "blast_scores", [shard], mybir.dt.float32, isOutput=False
    )
    s_ext = nc.declare_dram_parameter(
        "stage_labels", [shard], mybir.dt.int32, isOutput=False
    )
    out_ext = nc.declare_dram_parameter("out", [P, 2], mybir.dt.float32, isOutput=True)

    x_v = x_ext.ap().rearrange("(p f) -> p f", p=P)
    s_v = s_ext.ap().rearrange("(p f) -> p f", p=P)

    c47 = 4.0 / 7.0  # target step; folded into the Square's input scale
    c74 = 7.0 / 4.0  # x prescale so z = 7/4*(x - t) uses integer-exact v

    f32 = mybir.dt.float32
    bf16 = mybir.dt.bfloat16
    Alu = mybir.AluOpType
    Act = mybir.ActivationFunctionType

    with tile.TileContext(nc) as tc:
        with (
            tc.tile_pool(name="io", bufs=4) as io_pool,
            tc.tile_pool(name="mid", bufs=3) as mid_pool,
            tc.tile_pool(name="acc", bufs=1) as acc_pool,
        ):
            cnt_acc = acc_pool.tile([P, n_tiles], f32)
            sse_acc = acc_pool.tile([P, n_tiles], f32)
            red = acc_pool.tile([P, 2], f32)

            for k in range(n_tiles):
                x_t = io_pool.tile([P, fd], f32, tag="x")
                s_t = io_pool.tile([P, fd], mybir.dt.int32, tag="s")
                nc.sync.dma_start(out=x_t[:], in_=x_v[:, k * fd : (k + 1) * fd])
                nc.sync.dma_start(out=s_t[:], in_=s_v[:, k * fd : (k + 1) * fd])

                u = mid_pool.tile([P, fd], bf16, tag="u")
                m = mid_pool.tile([P, fd], bf16, tag="m")
                v = mid_pool.tile([P, fd], bf16, tag="v")
                z = mid_pool.tile([P, fd], bf16, tag="z")
                zm = mid_pool.tile([P, fd], bf16, tag="zm")
                sq = mid_pool.tile([P, fd], bf16, tag="sq")

                # GPSIMD (otherwise idle): u = min(s-7, 1); v = max(s-8, 0)
                nc.gpsimd.tensor_scalar(u[:], s_t[:], 7, 1, Alu.subtract, Alu.min)
                nc.gpsimd.tensor_scalar(v[:], s_t[:], 8, 0, Alu.subtract, Alu.max)
                # ACT: mask m = relu(u) in {0,1}; accumulate count for free
                nc.scalar.activation(
                    m[:], u[:], Act.Relu, accum_out=cnt_acc[:, k : k + 1]
                )
                # DVE: z = 7/4*x - v  (== 7/4*(x - target) since v = 7/4*t)
                nc.vector.scalar_tensor_tensor(
                    z[:], x_t[:], c74, v[:], Alu.mult, Alu.subtract
                )
                nc.vector.tensor_tensor(zm[:], z[:], m[:], Alu.mult)
                # ACT: sse += (4/7 * zm)^2 over masked elements
                nc.scalar.activation(
                    sq[:], zm[:], Act.Square, scale=c47,
                    accum_out=sse_acc[:, k : k + 1],
                )

            nc.vector.reduce_sum(red[:, 0:1], cnt_acc[:], axis=mybir.AxisListType.X)
            nc.vector.reduce_sum(red[:, 1:2], sse_acc[:], axis=mybir.AxisListType.X)
            nc.sync.dma_start(out=out_ext.ap()[:, :], in_=red[:])

    nc.finalize()
    return nc


def run(x, s, **spmd_kwargs):
    """Shard, run on 8 cores, host-reduce. Returns (loss, BassKernelResults)."""
    from concourse.bass_utils import run_bass_kernel_spmd

    if "nc" not in _NC_CACHE:
        _NC_CACHE["nc"] = build()
    nc = _NC_CACHE["nc"]

    in_maps = [
        {
            "blast_scores": x[i * SHARD : (i + 1) * SHARD],
            "stage_labels": s[i * SHARD : (i + 1) * SHARD],
        }
        for i in range(N_CORES)
    ]
    res = run_bass_kernel_spmd(nc, in_maps, core_ids=list(range(N_CORES)), **spmd_kwargs)

    tot = np.zeros(2, dtype=np.float64)
    for r in res.results:
        tot += r["out"].astype(np.float64).sum(axis=0)
    cnt, sse = tot[0], tot[1]
    val = sse / max(cnt, 1.0) if cnt > 0 else 0.0
    return np.asarray(val, dtype=np.float32), res


def kernel(**inputs):
    x = np.ascontiguousarray(np.asarray(inputs["blast_scores"], dtype=np.float32))
    s = np.ascontiguousarray(np.asarray(inputs["stage_labels"], dtype=np.int32))
    assert x.shape == (B,) and s.shape == (B,)
    return run(x, s)[0]


# revision 15
# speedup vs baseline: 7.6211x; 7.6211x over previous
"""Bass/Trainium2 kernel for nn_BlastocystAuxLoss.

Computes a masked MSE over B=16,777,216 elements:
    late stages are labels 8..15; target[s] = (s-8) * 4/7 for late stages;
    loss = sum_{s>=8} (x - target)^2 / count(s>=8)   (0.0 if count == 0)

Strategy: trivially data-parallel over 8 NeuronCores. Each core reads its
B/8 shard of blast_scores (f32) and stage_labels (i32) from HBM, computes
per-partition partial {count, sse} on-chip (DVE + ACT engines, bf16
elementwise math, f32 accumulation), and writes a [128, 2] partial-sums
tile. The final scalar reduction (8*128 partials -> sse/cnt) happens on
host in f64. No collectives needed.

Per-element identities used (s = label, x = score):
    mask  m = (s >= 8)
    target t = relu(s * 4/7 - 32/7)        (== (s-8)*4/7 clamped at 0)
    sse  += (m * (bf16(x) - t))^2          (m^2 == m)
    cnt  += m
"""

import numpy as np

B = 16777216
N_CORES = 8
SHARD = B // N_CORES  # 2,097,152
P = 128

_NC_CACHE = {}


def build(shard=SHARD, n_tiles=8):
    """Build the single-core Bass program (same SPMD program for all cores)."""
    import concourse.bacc as bacc
    import concourse.tile as tile
    from concourse import mybir

    free = shard // P
    fd = free // n_tiles
    assert fd * n_tiles * P == shard

    nc = bacc.Bacc("TRN2", target_bir_lowering=False)
    x_ext = nc.declare_dram_parameter(
        "blast_scores", [shard], mybir.dt.float32, isOutput=False
    )
    s_ext = nc.declare_dram_parameter(
        "stage_labels", [shard], mybir.dt.int32, isOutput=False
    )
    out_ext = nc.declare_dram_parameter("out", [P, 2], mybir.dt.float32, isOutput=True)

    x_v = x_ext.ap().rearrange("(p f) -> p f", p=P)
    s_v = s_ext.ap().rearrange("(p f) -> p f", p=P)

    c47 = 4.0 / 7.0  # target step; folded into the Square's input scale
    c74 = 7.0 / 4.0  # x prescale so z = 7/4*(x - t) uses integer-exact v

    f32 = mybir.dt.float32
    bf16 = mybir.dt.bfloat16
    Alu = mybir.AluOpType
    Act = mybir.ActivationFunctionType

    with tile.TileContext(nc) as tc:
        with (
            tc.tile_pool(name="io", bufs=4) as io_pool,
            tc.tile_pool(name="mid", bufs=3) as mid_pool,
            tc.tile_pool(name="acc", bufs=1) as acc_pool,
        ):
            cnt_acc = acc_pool.tile([P, n_tiles], f32)
            sse_acc = acc_pool.tile([P, n_tiles], f32)
            red = acc_pool.tile([P, 2], f32)
            # bias for the sigmoid step mask: m = sigmoid(64*s - 480)
            sig_bias = acc_pool.tile([P, 1], f32)
            nc.gpsimd.memset(sig_bias[:], -480.0)

            for k in range(n_tiles):
                x_t = io_pool.tile([P, fd], f32, tag="x")
                s_t = io_pool.tile([P, fd], mybir.dt.int32, tag="s")
                nc.sync.dma_start(out=x_t[:], in_=x_v[:, k * fd : (k + 1) * fd])
                nc.sync.dma_start(out=s_t[:], in_=s_v[:, k * fd : (k + 1) * fd])

                m = mid_pool.tile([P, fd], bf16, tag="m")
                v = mid_pool.tile([P, fd], bf16, tag="v")
                z = mid_pool.tile([P, fd], bf16, tag="z")
                zm = mid_pool.tile([P, fd], bf16, tag="zm")
                sq = mid_pool.tile([P, fd], bf16, tag="sq")

                # ACT: step mask m = sigmoid(64*(s - 7.5)) in {0,1} exactly
                # (saturated at +-32); accumulate count for free
                nc.scalar.activation(
                    m[:], s_t[:], Act.Sigmoid, bias=sig_bias[:], scale=64.0,
                    accum_out=cnt_acc[:, k : k + 1],
                )
                # DVE: v = max(s-8, 0)
                nc.vector.tensor_scalar(v[:], s_t[:], 8, 0, Alu.subtract, Alu.max)
                # DVE: z = 7/4*x - v  (== 7/4*(x - target) since v = 7/4*t)
                nc.vector.scalar_tensor_tensor(
                    z[:], x_t[:], c74, v[:], Alu.mult, Alu.subtract
                )
                nc.vector.tensor_tensor(zm[:], z[:], m[:], Alu.mult)
                # ACT: sse += (4/7 * zm)^2 over masked elements
                nc.scalar.activation(
                    sq[:], zm[:], Act.Square, scale=c47,
                    accum_out=sse_acc[:, k : k + 1],
                )

            nc.vector.reduce_sum(red[:, 0:1], cnt_acc[:], axis=mybir.AxisListType.X)
            nc.vector.reduce_sum(red[:, 1:2], sse_acc[:], axis=mybir.AxisListType.X)
            nc.sync.dma_start(out=out_ext.ap()[:, :], in_=red[:])

    nc.finalize()
    return nc


def run(x, s, **spmd_kwargs):
    """Shard, run on 8 cores, host-reduce. Returns (loss, BassKernelResults)."""
    from concourse.bass_utils import run_bass_kernel_spmd

    if "nc" not in _NC_CACHE:
        _NC_CACHE["nc"] = build()
    nc = _NC_CACHE["nc"]

    in_maps = [
        {
            "blast_scores": x[i * SHARD : (i + 1) * SHARD],
            "stage_labels": s[i * SHARD : (i + 1) * SHARD],
        }
        for i in range(N_CORES)
    ]
    res = run_bass_kernel_spmd(nc, in_maps, core_ids=list(range(N_CORES)), **spmd_kwargs)

    tot = np.zeros(2, dtype=np.float64)
    for r in res.results:
        tot += r["out"].astype(np.float64).sum(axis=0)
    cnt, sse = tot[0], tot[1]
    val = sse / max(cnt, 1.0) if cnt > 0 else 0.0
    return np.asarray(val, dtype=np.float32), res


def kernel(**inputs):
    x = np.ascontiguousarray(np.asarray(inputs["blast_scores"], dtype=np.float32))
    s = np.ascontiguousarray(np.asarray(inputs["stage_labels"], dtype=np.int32))
    assert x.shape == (B,) and s.shape == (B,)
    return run(x, s)[0]


# revision 17
# speedup vs baseline: 7.6277x; 1.0009x over previous
"""Bass/Trainium2 kernel for nn_BlastocystAuxLoss.

Computes a masked MSE over B=16,777,216 elements:
    late stages are labels 8..15; target[s] = (s-8) * 4/7 for late stages;
    loss = sum_{s>=8} (x - target)^2 / count(s>=8)   (0.0 if count == 0)

Strategy: trivially data-parallel over 8 NeuronCores. Each core reads its
B/8 shard of blast_scores (f32) and stage_labels (i32) from HBM, computes
per-partition partial {count, sse} on-chip (DVE + ACT engines, bf16
elementwise math, f32 accumulation), and writes a [128, 2] partial-sums
tile. The final scalar reduction (8*128 partials -> sse/cnt) happens on
host in f64. No collectives needed.

Per-element identities used (s = label, x = score):
    mask  m = (s >= 8)
    target t = relu(s * 4/7 - 32/7)        (== (s-8)*4/7 clamped at 0)
    sse  += (m * (bf16(x) - t))^2          (m^2 == m)
    cnt  += m
"""

from contextlib import ExitStack

import numpy as np

B = 16777216
N_CORES = 8
SHARD = B // N_CORES  # 2,097,152
P = 128

_NC_CACHE = {}


def build(shard=SHARD, n_tiles=8):
    """Build the single-core Bass program (same SPMD program for all cores)."""
    import concourse.bacc as bacc
    import concourse.tile as tile
    from concourse import mybir

    free = shard // P
    fd = free // n_tiles
    assert fd * n_tiles * P == shard

    nc = bacc.Bacc("TRN2", target_bir_lowering=False)
    x_ext = nc.declare_dram_parameter(
        "blast_scores", [shard], mybir.dt.float32, isOutput=False
    )
    s_ext = nc.declare_dram_parameter(
        "stage_labels", [shard], mybir.dt.int32, isOutput=False
    )
    out_ext = nc.declare_dram_parameter("out", [P, 2], mybir.dt.float32, isOutput=True)

    x_v = x_ext.ap().rearrange("(p f) -> p f", p=P)
    s_v = s_ext.ap().rearrange("(p f) -> p f", p=P)

    c47 = 4.0 / 7.0  # target step; folded into the Square's input scale
    c74 = 7.0 / 4.0  # x prescale so z = 7/4*(x - t) uses integer-exact v

    f32 = mybir.dt.float32
    bf16 = mybir.dt.bfloat16
    Alu = mybir.AluOpType
    Act = mybir.ActivationFunctionType

    with tile.TileContext(nc) as tc:
        with (
            tc.tile_pool(name="io", bufs=4) as io_pool,
            tc.tile_pool(name="mid", bufs=3) as mid_pool,
            tc.tile_pool(name="acc", bufs=1) as acc_pool,
        ):
            cnt_acc = acc_pool.tile([P, n_tiles], f32)
            sse_acc = acc_pool.tile([P, n_tiles], f32)
            red = acc_pool.tile([P, 2], f32)
            # bias for the sigmoid step mask: m = sigmoid(64*s - 480)
            sig_bias = acc_pool.tile([P, 1], f32)
            nc.gpsimd.memset(sig_bias[:], -480.0)

            for k in range(n_tiles):
                x_t = io_pool.tile([P, fd], f32, tag="x")
                s_t = io_pool.tile([P, fd], mybir.dt.int32, tag="s")
                nc.sync.dma_start(out=x_t[:], in_=x_v[:, k * fd : (k + 1) * fd])
                nc.sync.dma_start(out=s_t[:], in_=s_v[:, k * fd : (k + 1) * fd])

                m = mid_pool.tile([P, fd], bf16, tag="m")
                v = mid_pool.tile([P, fd], bf16, tag="v")
                z = mid_pool.tile([P, fd], bf16, tag="z")
                zm = mid_pool.tile([P, fd], bf16, tag="zm")
                sq = mid_pool.tile([P, fd], bf16, tag="sq")

                # ACT: step mask m = sigmoid(64*(s - 7.5)) in {0,1} exactly
                # (saturated at +-32); accumulate count for free
                nc.scalar.activation(
                    m[:], s_t[:], Act.Sigmoid, bias=sig_bias[:], scale=64.0,
                    accum_out=cnt_acc[:, k : k + 1],
                )
                # DVE: v = max(s-8, 0)
                nc.vector.tensor_scalar(v[:], s_t[:], 8, 0, Alu.subtract, Alu.max)
                # DVE: z = 7/4*x - v  (== 7/4*(x - target) since v = 7/4*t)
                nc.vector.scalar_tensor_tensor(
                    z[:], x_t[:], c74, v[:], Alu.mult, Alu.subtract
                )
                nc.vector.tensor_tensor(zm[:], z[:], m[:], Alu.mult)
                # ACT: sse += (4/7 * zm)^2 over masked elements
                nc.scalar.activation(
                    sq[:], zm[:], Act.Square, scale=c47,
                    accum_out=sse_acc[:, k : k + 1],
                )

            nc.vector.reduce_sum(red[:, 0:1], cnt_acc[:], axis=mybir.AxisListType.X)
            nc.vector.reduce_sum(red[:, 1:2], sse_acc[:], axis=mybir.AxisListType.X)
            nc.sync.dma_start(out=out_ext.ap()[:, :], in_=red[:])

    nc.finalize()
    return nc


def build_raw(shard=2097152, n_tiles=8, ring=4):
    import concourse.bacc as bacc
    from concourse import mybir

    free = shard // P
    fd = free // n_tiles
    assert fd * n_tiles * P == shard

    nc = bacc.Bacc("TRN2", target_bir_lowering=False)
    x_ext = nc.declare_dram_parameter(
        "blast_scores", [shard], mybir.dt.float32, isOutput=False
    )
    s_ext = nc.declare_dram_parameter(
        "stage_labels", [shard], mybir.dt.int32, isOutput=False
    )
    out_ext = nc.declare_dram_parameter("out", [P, 2], mybir.dt.float32, isOutput=True)

    x_v = x_ext.ap().rearrange("(p f) -> p f", p=P)
    s_v = s_ext.ap().rearrange("(p f) -> p f", p=P)

    c47 = 4.0 / 7.0
    c74 = 7.0 / 4.0

    f32 = mybir.dt.float32
    i32 = mybir.dt.int32
    bf16 = mybir.dt.bfloat16
    Alu = mybir.AluOpType
    Act = mybir.ActivationFunctionType

    R = ring
    NT = n_tiles

    x_t = [nc.alloc_sbuf_tensor(f"x{i}", [P, fd], f32).ap() for i in range(R)]
    s_t = [nc.alloc_sbuf_tensor(f"s{i}", [P, fd], i32).ap() for i in range(R)]
    m_t = [nc.alloc_sbuf_tensor(f"m{i}", [P, fd], bf16).ap() for i in range(R)]
    v_t = [nc.alloc_sbuf_tensor(f"v{i}", [P, fd], bf16).ap() for i in range(2)]
    z_t = [nc.alloc_sbuf_tensor(f"z{i}", [P, fd], bf16).ap() for i in range(2)]
    zm_t = [nc.alloc_sbuf_tensor(f"zm{i}", [P, fd], bf16).ap() for i in range(R)]
    sq_t = nc.alloc_sbuf_tensor("sq", [P, fd], bf16).ap()
    cnt_acc = nc.alloc_sbuf_tensor("cnt_acc", [P, NT], f32).ap()
    sse_acc = nc.alloc_sbuf_tensor("sse_acc", [P, NT], f32).ap()
    red = nc.alloc_sbuf_tensor("red", [P, 2], f32).ap()
    sig_bias = nc.alloc_sbuf_tensor("sig_bias", [P, 1], f32).ap()

    with ExitStack() as ctx:
        dma_x = [ctx.enter_context(nc.semaphore(f"dma_x{i}")) for i in range(R)]
        dma_s = [ctx.enter_context(nc.semaphore(f"dma_s{i}")) for i in range(R)]
        dve = ctx.enter_context(nc.semaphore("dve"))
        act = ctx.enter_context(nc.semaphore("act"))
        outd = ctx.enter_context(nc.semaphore("outd"))
        bias_rdy = ctx.enter_context(nc.semaphore("bias_rdy"))
        block = ctx.enter_context(nc.Block())

        # Per-tile semaphore increments:
        #   DVE: 3 per tile (v, z, zm), then 2 final reduces
        #   ACT: 2 per tile (m, sq)
        #   DMA slot sems: +16 per transfer into that slot

        @block.sync
        def _(sync):
            for k in range(NT):
                i = k % R
                if k >= R:
                    # x slot free when z(k-R) done; s slot free when
                    # v(k-R) (implied by z) and m(k-R) done
                    sync.wait_ge(dve, 3 * (k - R) + 2)
                    sync.wait_ge(act, 2 * (k - R) + 1)
                sync.dma_start(
                    out=x_t[i][:, :], in_=x_v[:, k * fd : (k + 1) * fd]
                ).then_inc(dma_x[i], 16)
                sync.dma_start(
                    out=s_t[i][:, :], in_=s_v[:, k * fd : (k + 1) * fd]
                ).then_inc(dma_s[i], 16)
            sync.wait_ge(dve, 3 * NT + 2)
            sync.dma_start(out=out_ext.ap()[:, :], in_=red[:, :]).then_inc(outd, 16)
            sync.wait_ge(outd, 16)

        @block.vector
        def _(vector):
            vector.memset(sig_bias[:, :], -480.0).then_inc(bias_rdy, 1)
            for k in range(NT):
                i = k % R
                rnd = 16 * (k // R + 1)
                # v = max(s-8, 0)
                vector.wait_ge(dma_s[i], rnd)
                vector.tensor_scalar(
                    v_t[k % 2][:, :], s_t[i][:, :], 8, 0, Alu.subtract, Alu.max
                ).then_inc(dve, 1)
                # z = 7/4*x - v
                vector.wait_ge(dma_x[i], rnd)
                vector.wait_ge(dve, 3 * k + 1)  # v(k) drained
                vector.scalar_tensor_tensor(
                    z_t[k % 2][:, :], x_t[i][:, :], c74, v_t[k % 2][:, :],
                    Alu.mult, Alu.subtract,
                ).then_inc(dve, 1)
                # zm = z * m   (m(k) ready when act >= 2k+1)
                vector.wait_ge(act, 2 * k + 1)
                vector.wait_ge(dve, 3 * k + 2)  # z(k) drained
                vector.tensor_tensor(
                    zm_t[i][:, :], z_t[k % 2][:, :], m_t[i][:, :], Alu.mult
                ).then_inc(dve, 1)
            vector.wait_ge(act, 2 * NT)  # all sq done -> sse_acc complete
            vector.reduce_sum(
                red[:, 0:1], cnt_acc[:, :], axis=mybir.AxisListType.X
            ).then_inc(dve, 1)
            vector.reduce_sum(
                red[:, 1:2], sse_acc[:, :], axis=mybir.AxisListType.X
            ).then_inc(dve, 1)

        @block.scalar
        def _(scalar):
            scalar.wait_ge(bias_rdy, 1)
            for k in range(NT):
                i = k % R
                rnd = 16 * (k // R + 1)
                # m = sigmoid(64*s - 480); cnt accum
                scalar.wait_ge(dma_s[i], rnd)
                if k >= R:
                    # m slot free when zm(k-R) done
                    scalar.wait_ge(dve, 3 * (k - R) + 3)
                scalar.activation(
                    m_t[i][:, :], s_t[i][:, :], Act.Sigmoid,
                    bias=sig_bias[:, :], scale=64.0,
                    accum_out=cnt_acc[:, k : k + 1],
                ).then_inc(act, 1)
                # sq = Square(zm * 4/7); sse accum; zm(k): dve >= 3k+3
                scalar.wait_ge(dve, 3 * k + 3)
                scalar.activation(
                    sq_t[:, :], zm_t[i][:, :], Act.Square, scale=c47,
                    accum_out=sse_acc[:, k : k + 1],
                ).then_inc(act, 1)

    nc.finalize()
    return nc


def run(x, s, **spmd_kwargs):
    """Shard, run on 8 cores, host-reduce. Returns (loss, BassKernelResults)."""
    from concourse.bass_utils import run_bass_kernel_spmd

    if "nc" not in _NC_CACHE:
        _NC_CACHE["nc"] = build_raw()
    nc = _NC_CACHE["nc"]

    in_maps = [
        {
            "blast_scores": x[i * SHARD : (i + 1) * SHARD],
            "stage_labels": s[i * SHARD : (i + 1) * SHARD],
        }
        for i in range(N_CORES)
    ]
    res = run_bass_kernel_spmd(nc, in_maps, core_ids=list(range(N_CORES)), **spmd_kwargs)

    tot = np.zeros(2, dtype=np.float64)
    for r in res.results:
        tot += r["out"].astype(np.float64).sum(axis=0)
    cnt, sse = tot[0], tot[1]
    val = sse / max(cnt, 1.0) if cnt > 0 else 0.0
    return np.asarray(val, dtype=np.float32), res


def kernel(**inputs):
    x = np.ascontiguousarray(np.asarray(inputs["blast_scores"], dtype=np.float32))
    s = np.ascontiguousarray(np.asarray(inputs["stage_labels"], dtype=np.int32))
    assert x.shape == (B,) and s.shape == (B,)
    return run(x, s)[0]
